# revision 1
# baseline (speedup 1.0000x reference)
"""Trainium2 Bass kernel for nn_CartesianPlaneEmbeddingNetwork (embedding_lookup).

Math (faithful to the reference, including its xz-from-plane_xy quirk):
    p0   = plane_xy[0]                                   (128, 256, 256) f32
    xy   = bilinear(p0, x, y); xz = bilinear(p0, x, z)   per point
    feat = xy * xz * xz
    out  = (sin(30*(feat@W1.T+b1)) -> sin(30*(.@W2.T+b2)) -> @W3.T+b3)

Strategy (8 NeuronCores, data-parallel over points):
  * Host: re-layout p0 into an xp-major patch table T[xp, y, h, c] =
    p0[c, y, 2*xp+h] (xp = column pair, h in {0,1}).  One 2KB row-pair
    (y, y+1 at fixed xp) is then a full 2x2 bilinear patch = ONE DMA
    descriptor.  dma_gather throughput is descriptor-limited (~8.7ns/desc),
    so 2KB/desc doubles gather bandwidth vs 1KB/desc.
  * Odd x0 points need patches straddling column pairs: a second shifted
    table To[xp, y, h, c] = p0[c, y, 2*xp+1+h].  Host splits each core's
    62500 points by parity of x0; patch index xp*256+y0 <= 32766 fits the
    gather's int16 indices.
  * Per batch (4 point-tiles = 512 points, 1024 idxs, 2 alternating SWDGE
    queues): gather -> [128 pts, 8 slots, 512] SBUF; DVE applies bilinear
    corner weights via broadcast-AP multiplies + add-trees; ScalarE
    squares; TensorE transposes features and runs the MLP (Sin from PSUM).
  * Host: gathers per-core outputs, undoes the parity permutation.
"""

import numpy as np

import concourse.bass as bass
import concourse.bacc as bacc
import concourse.mybir as mybir
import concourse.tile as tile
from concourse.masks import make_identity

N_CORES = 8
N_TOTAL = 500_000
N_PER_CORE = N_TOTAL // N_CORES          # 62500
GB = 4                                   # point-tiles per gather batch
TPG = 252                                # point-tiles per parity group (cap 32256 ≈ +8σ)
PTS_PER_GROUP = TPG * 128                # 32256
OUT_PER_CORE = 2 * PTS_PER_GROUP         # 64512
NUM_IDX = GB * 2 * 128                   # 1024 idxs per dma_gather (2 patches/point)
ELEM = 512                               # f32 per gathered element (2x2 patch = 2KB)
STEP = 256                               # f32 between consecutive patch rows (1KB)
TAB_ROWS = 128 * 256                     # (xp, y) patch rows

F32 = mybir.dt.float32
F32R = mybir.dt.float32r
I16 = mybir.dt.int16


def build_nc(tpg=TPG, n_queues=2):
    assert tpg % GB == 0
    nb = tpg // GB
    nc = bacc.Bacc("TRN2", target_bir_lowering=False, debug=False,
                   enable_asserts=False, num_devices=N_CORES,
                   num_swdge_queues=n_queues)

    tab_d = {g: nc.dram_tensor(f"tab_{g}", [TAB_ROWS * STEP], F32,
                               kind="ExternalInput") for g in "eo"}
    idx_d = {g: nc.dram_tensor(f"idx_{g}", [128, nb * (NUM_IDX // 16)], I16,
                               kind="ExternalInput") for g in "eo"}
    w_d = {g: nc.dram_tensor(f"w_{g}", [128, tpg * 8], F32, kind="ExternalInput")
           for g in "eo"}
    w1t_d = nc.dram_tensor("w1t", [128, 128], F32R, kind="ExternalInput")
    w2t_d = nc.dram_tensor("w2t", [128, 128], F32R, kind="ExternalInput")
    w3t_d = nc.dram_tensor("w3t", [128, 1], F32R, kind="ExternalInput")
    b1s_d = nc.dram_tensor("b1s", [128, 1], F32, kind="ExternalInput")
    b2s_d = nc.dram_tensor("b2s", [128, 1], F32, kind="ExternalInput")
    out_d = nc.dram_tensor("out", [2 * tpg * 128], F32, kind="ExternalOutput")

    Sin = mybir.ActivationFunctionType.Sin
    mult = mybir.AluOpType.mult
    IC = NUM_IDX // 16                    # idx tile cols per batch

    with tile.TileContext(nc) as tc:
        with (
            tc.tile_pool(name="const", bufs=1) as cpool,
            tc.tile_pool(name="work", bufs=2) as wpool,
            tc.tile_pool(name="gather", bufs=4) as gpool,
            tc.tile_pool(name="ps_ft", bufs=2, space="PSUM") as ps_ft_pool,
            tc.tile_pool(name="ps_mm", bufs=4, space="PSUM") as ps_mm_pool,
        ):
            ident = cpool.tile([128, 128], F32, tag="ident")
            make_identity(nc, ident[:])

            def load_const(name, dram, shape, dtype):
                t = cpool.tile(shape, dtype, tag=name)
                nc.sync.dma_start(out=t[:], in_=dram.ap())
                return t

            w1t_s = load_const("w1t", w1t_d, [128, 128], F32R)
            w2t_s = load_const("w2t", w2t_d, [128, 128], F32R)
            w3t_s = load_const("w3t", w3t_d, [128, 1], F32R)
            b1s_s = load_const("b1s", b1s_d, [128, 1], F32)
            b2s_s = load_const("b2s", b2s_d, [128, 1], F32)
            idx_s = {g: load_const(f"idx_{g}", idx_d[g], [128, nb * IC], I16)
                     for g in "eo"}
            w_s = {g: load_const(f"w_{g}", w_d[g], [128, tpg * 8], F32)
                   for g in "eo"}

            # Overlapping row view: row r = floats [r*256, r*256+512)
            tab_view = {g: bass.AP(tab_d[g].ap().tensor, 0,
                                   [[STEP, TAB_ROWS - 1], [1, ELEM]])
                        for g in "eo"}

            qn = 0
            for gi, g in enumerate("eo"):
                goff = gi * tpg * 128
                for b in range(nb):
                    gt = gpool.tile([128, GB * 2, ELEM], F32, tag="gt")
                    nc.gpsimd.dma_gather(
                        gt[:], tab_view[g],
                        idx_s[g][:, b * IC:(b + 1) * IC],
                        NUM_IDX, NUM_IDX, ELEM, elem_step=STEP,
                        single_packet=False, queue_num=qn % n_queues,
                    )
                    qn += 1
                    # [128, tile(4), block(8 = y0a y0b y1a y1b | z...), 128]
                    gv = gt[:].rearrange("p (t s) (h e) -> p t (s h) e",
                                         t=GB, h=4)
                    wb = w_s[g][:, b * 32:(b + 1) * 32].rearrange(
                        "p (t j) -> p t j", t=GB)
                    w8 = wb.unsqueeze(3).to_broadcast([128, GB, 8, 128])

                    p8 = wpool.tile([128, GB * 8 * 128], F32, tag="p8")
                    p8v = p8[:].rearrange("p (t j e) -> p t j e", t=GB, j=8)
                    nc.vector.tensor_tensor(out=p8v, in0=gv, in1=w8, op=mult)
                    # pairwise tree: (j0+j1),(j2+j3),... -> then again
                    q4 = wpool.tile([128, GB * 4 * 128], F32, tag="q4")
                    q4v = q4[:].rearrange("p (t j e) -> p t j e", t=GB, j=4)
                    nc.vector.tensor_add(q4v, p8v[:, :, 0::2, :], p8v[:, :, 1::2, :])
                    q2 = wpool.tile([128, GB * 2 * 128], F32, tag="q2")
                    q2v = q2[:].rearrange("p (t j e) -> p t j e", t=GB, j=2)
                    nc.vector.tensor_add(q2v, q4v[:, :, 0::2, :], q4v[:, :, 1::2, :])
                    xy = q2v[:, :, 0, :]                       # [128, GB, 128]
                    xz = q2v[:, :, 1, :]

                    xz2 = wpool.tile([128, GB * 128], F32, tag="xz2")
                    xz2v = xz2[:].rearrange("p (t e) -> p t e", t=GB)
                    nc.scalar.square(xz2v, xz)
                    feat = wpool.tile([128, GB * 128], F32, tag="feat")
                    featv = feat[:].rearrange("p (t e) -> p t e", t=GB)
                    nc.vector.tensor_tensor(out=featv, in0=xy, in1=xz2v, op=mult)

                    ps_ft = ps_ft_pool.tile([128, GB * 128], F32, tag="ft")
                    for t in range(GB):
                        nc.tensor.transpose(
                            ps_ft[:, t * 128:(t + 1) * 128],
                            feat[:, t * 128:(t + 1) * 128],
                            ident[:],
                        )
                    ft = wpool.tile([128, GB * 128], F32R, tag="ft_s")
                    nc.scalar.copy(ft[:], ps_ft[:])

                    ps1 = ps_mm_pool.tile([128, GB * 128], F32, tag="mm")
                    nc.tensor.matmul(ps1[:], w1t_s[:], ft[:], start=True, stop=True)
                    h1 = wpool.tile([128, GB * 128], F32R, tag="h1")
                    nc.scalar.activation(h1[:], ps1[:], Sin, bias=b1s_s[:], scale=30.0)

                    ps2 = ps_mm_pool.tile([128, GB * 128], F32, tag="mm")
                    nc.tensor.matmul(ps2[:], w2t_s[:], h1[:], start=True, stop=True)
                    h2 = wpool.tile([128, GB * 128], F32R, tag="h2")
                    nc.scalar.activation(h2[:], ps2[:], Sin, bias=b2s_s[:], scale=30.0)

                    ps3 = ps_mm_pool.tile([1, GB * 128], F32, tag="mm")
                    nc.tensor.matmul(ps3[:], w3t_s[:], h2[:], start=True, stop=True)
                    ob = wpool.tile([1, GB * 128], F32, tag="ob")
                    nc.scalar.copy(ob[:], ps3[:])
                    nc.sync.dma_start(
                        out=out_d.ap()[goff + b * 512: goff + (b + 1) * 512],
                        in_=ob[:],
                    )
    nc.compile()
    return nc


def prep_core(pts, tpg=TPG):
    """Host-side prep for one core's points -> device input tensors + orders."""
    nb = tpg // GB
    cap = tpg * 128
    IC = NUM_IDX // 16
    gx, gy, gz = pts[:, 0], pts[:, 1], pts[:, 2]
    ix = (gx + np.float32(1.0)) * np.float32(127.5)
    iy = (gy + np.float32(1.0)) * np.float32(127.5)
    iz = (gz + np.float32(1.0)) * np.float32(127.5)
    x0 = np.clip(np.floor(ix), 0, 254).astype(np.int32)
    y0 = np.clip(np.floor(iy), 0, 254).astype(np.int32)
    z0 = np.clip(np.floor(iz), 0, 254).astype(np.int32)
    wx1 = (ix - x0).astype(np.float32)
    wy1 = (iy - y0).astype(np.float32)
    wz1 = (iz - z0).astype(np.float32)
    wx0 = np.float32(1.0) - wx1
    wy0 = np.float32(1.0) - wy1
    wz0 = np.float32(1.0) - wz1

    par = (x0 & 1).astype(bool)
    # Sort each group by y-window patch index: descriptors then walk the
    # table near-sequentially (z-windows share the xp block), which lifts
    # SDMA gather throughput from random-read (~230GB/s) toward sequential.
    ykey = (x0 >> 1) * 256 + y0
    orders = []
    for mask in (~par, par):
        o = np.nonzero(mask)[0]
        orders.append(o[np.argsort(ykey[o], kind="stable")])
    outm = {}
    for gname, order in zip("eo", orders):
        ne = len(order)
        assert ne <= cap, f"group {gname} overflow: {ne} > {cap}"
        idx2 = np.zeros((cap, 2), np.int32)
        base = (x0[order] >> 1) * 256
        idx2[:ne, 0] = base + y0[order]
        idx2[:ne, 1] = base + z0[order]
        assert idx2.max() <= 32766
        w8 = np.zeros((cap, 8), np.float32)
        w8[:ne] = np.stack([
            wy0[order] * wx0[order], wy0[order] * wx1[order],
            wy1[order] * wx0[order], wy1[order] * wx1[order],
            wz0[order] * wx0[order], wz0[order] * wx1[order],
            wz1[order] * wx0[order], wz1[order] * wx1[order],
        ], axis=1)
        # idx stream -> [128, nb*IC] int16 tile (16-partition wrap, 8x replicated)
        a = idx2.reshape(nb, GB, 128, 2).transpose(0, 1, 3, 2)   # (b, t, s, p)
        stream = a.reshape(nb, NUM_IDX)
        t16 = stream.reshape(nb, NUM_IDX // 16, 16).transpose(0, 2, 1)  # (b, q, s)
        idx_dev = np.concatenate(
            [np.tile(t16[b], (8, 1)) for b in range(nb)], axis=1
        ).astype(np.int16)
        w_dev = np.ascontiguousarray(
            w8.reshape(tpg, 128, 8).transpose(1, 0, 2).reshape(128, tpg * 8))
        outm[f"idx_{gname}"] = idx_dev
        outm[f"w_{gname}"] = w_dev
        outm[f"ord_{gname}"] = order
    return outm


def prep_shared(inputs):
    plane = np.asarray(inputs["plane_xy"], np.float32)[0]        # (C, H, W)
    hwc = np.ascontiguousarray(plane.transpose(1, 2, 0))         # (H, W, C)
    # T[xp, y, h, c] = p0[c, y, 2xp+h]
    tab_e = np.ascontiguousarray(
        hwc.reshape(256, 128, 2, 128).transpose(1, 0, 2, 3)).reshape(-1)
    hwc_s = np.concatenate([hwc[:, 1:, :],
                            np.zeros((256, 1, 128), np.float32)], axis=1)
    tab_o = np.ascontiguousarray(
        hwc_s.reshape(256, 128, 2, 128).transpose(1, 0, 2, 3)).reshape(-1)
    shared = {
        "tab_e": tab_e,
        "tab_o": tab_o,
        "w1t": np.ascontiguousarray(np.asarray(inputs["W1"], np.float32).T),
        "w2t": np.ascontiguousarray(np.asarray(inputs["W2"], np.float32).T),
        "w3t": np.ascontiguousarray(np.asarray(inputs["W3"], np.float32).T),
        "b1s": (np.float32(30.0) * np.asarray(inputs["b1"], np.float32)).reshape(128, 1),
        "b2s": (np.float32(30.0) * np.asarray(inputs["b2"], np.float32)).reshape(128, 1),
    }
    return shared


_NC_CACHE = {}


def get_nc(tpg=TPG):
    if tpg not in _NC_CACHE:
        _NC_CACHE[tpg] = build_nc(tpg)
    return _NC_CACHE[tpg]


LAST_RESULT = None


def kernel(_trace=False, **inputs):
    global LAST_RESULT
    from concourse.bass_utils import run_bass_kernel_spmd

    coords = np.asarray(inputs["coordinates"], np.float32).reshape(-1, 3)
    n = coords.shape[0]
    assert n == N_TOTAL, n
    shared = prep_shared(inputs)
    b3 = np.float32(np.asarray(inputs["b3"], np.float32).reshape(-1)[0])
    nc = get_nc()
    in_maps, metas = [], []
    for ci in range(N_CORES):
        pts = coords[ci * N_PER_CORE:(ci + 1) * N_PER_CORE]
        m = prep_core(pts)
        metas.append(m)
        in_maps.append({**shared,
                        "idx_e": m["idx_e"], "w_e": m["w_e"],
                        "idx_o": m["idx_o"], "w_o": m["w_o"]})
    res = run_bass_kernel_spmd(nc, in_maps, core_ids=list(range(N_CORES)),
                               trace=_trace)
    LAST_RESULT = res
    outs = []
    for ci in range(N_CORES):
        r = np.asarray(res.results[ci]["out"], np.float32)
        m = metas[ci]
        full = np.empty(N_PER_CORE, np.float32)
        full[m["ord_e"]] = r[:len(m["ord_e"])]
        full[m["ord_o"]] = r[PTS_PER_GROUP:PTS_PER_GROUP + len(m["ord_o"])]
        outs.append(full + b3)
    return np.concatenate(outs).reshape(1, N_TOTAL, 1)



# revision 5
# speedup vs baseline: 1.5360x; 1.5360x over previous
"""Trainium2 Bass kernel for nn_CartesianPlaneEmbeddingNetwork (embedding_lookup).

Math (faithful to the reference, including its xz-from-plane_xy quirk):
    p0   = plane_xy[0]                                   (128, 256, 256) f32
    xy   = bilinear(p0, x, y); xz = bilinear(p0, x, z)   per point
    feat = xy * xz * xz
    out  = (sin(30*(feat@W1.T+b1)) -> sin(30*(.@W2.T+b2)) -> @W3.T+b3)

Strategy (8 NeuronCores, data-parallel over points):
  * Host: re-layout p0 into an xp-major patch table T[xp, y, h, c] =
    p0[c, y, 2*xp+h] (xp = column pair, h in {0,1}), stored in fp16.
    One row-pair (y, y+1 at fixed xp) is a full 2x2 bilinear patch =
    ONE 1KB DMA descriptor (fp16 halves gather bytes vs f32; DMA
    engines are byte-rate-limited at >=512B/descriptor).
  * Odd x0 points use a second shifted table To[xp, y, h, c] =
    p0[c, y, 2*xp+1+h].  Host splits each core's 62500 points by parity
    of x0; patch index xp*256+y0 <= 32766 fits int16 gather indices.
  * Gathers are issued prepare_only + trigger_dma so the Pool engine
    only generates descriptors; transfers stream back-to-back through
    the 16 DMA engines instead of blocking the engine per gather.
  * Per batch (4 point-tiles = 512 points, 1024 idxs, 2 SWDGE queues):
    DVE applies bilinear corner weights in fp16 (2 elem/cycle packed
    mode; weights host-duplicated in adjacent pairs so every operand
    keeps a stride-1 16-bit pair in its innermost AP dim) + add-trees;
    ScalarE squares; TensorE transposes features and runs the MLP in
    fp16 (f32 PSUM accumulation), Sin applied from PSUM.
  * Host: gathers per-core outputs, undoes the parity permutation.
"""

import numpy as np

import concourse.bass as bass
import concourse.bacc as bacc
import concourse.mybir as mybir
import concourse.tile as tile
from concourse.masks import make_identity

N_CORES = 8
N_TOTAL = 500_000
N_PER_CORE = N_TOTAL // N_CORES          # 62500
GB = 4                                   # point-tiles per gather batch
TPG = 252                                # point-tiles per parity group (cap 32256 ≈ +8σ)
PTS_PER_GROUP = TPG * 128                # 32256
OUT_PER_CORE = 2 * PTS_PER_GROUP         # 64512
NUM_IDX = GB * 2 * 128                   # 1024 idxs per dma_gather (2 patches/point)
ELEM = 512                               # f16 per gathered element (2x2 patch = 1KB)
STEP = 256                               # f16 between consecutive patch rows (512B)
TAB_ROWS = 128 * 256                     # (xp, y) patch rows

F32 = mybir.dt.float32
F16 = mybir.dt.float16
I16 = mybir.dt.int16

PREP_TRIGGER = False


def build_nc(tpg=TPG, n_queues=2):
    assert tpg % GB == 0
    nb = tpg // GB
    nc = bacc.Bacc("TRN2", target_bir_lowering=False, debug=False,
                   enable_asserts=False, num_devices=N_CORES,
                   num_swdge_queues=n_queues)

    tab_d = {g: nc.dram_tensor(f"tab_{g}", [TAB_ROWS * STEP], F16,
                               kind="ExternalInput") for g in "eo"}
    idx_d = {g: nc.dram_tensor(f"idx_{g}", [128, nb * (NUM_IDX // 16)], I16,
                               kind="ExternalInput") for g in "eo"}
    # weights duplicated in adjacent pairs: [...,(slot, 2)] so fp16 2x packing
    # sees a stride-1 pair in the innermost dim
    w_d = {g: nc.dram_tensor(f"w_{g}", [128, tpg * 16], F16, kind="ExternalInput")
           for g in "eo"}
    w1t_d = nc.dram_tensor("w1t", [128, 128], F16, kind="ExternalInput")
    w2t_d = nc.dram_tensor("w2t", [128, 128], F16, kind="ExternalInput")
    w3t_d = nc.dram_tensor("w3t", [128, 1], F16, kind="ExternalInput")
    b1s_d = nc.dram_tensor("b1s", [128, 1], F32, kind="ExternalInput")
    b2s_d = nc.dram_tensor("b2s", [128, 1], F32, kind="ExternalInput")
    out_d = nc.dram_tensor("out", [2 * tpg * 128], F32, kind="ExternalOutput")

    Sin = mybir.ActivationFunctionType.Sin
    mult = mybir.AluOpType.mult
    IC = NUM_IDX // 16                    # idx tile cols per batch

    with tile.TileContext(nc) as tc:
        with (
            tc.tile_pool(name="const", bufs=1) as cpool,
            tc.tile_pool(name="work", bufs=2) as wpool,
            tc.tile_pool(name="gather", bufs=4) as gpool,
            tc.tile_pool(name="ps_ft", bufs=2, space="PSUM") as ps_ft_pool,
            tc.tile_pool(name="ps_mm", bufs=4, space="PSUM") as ps_mm_pool,
        ):
            ident = cpool.tile([128, 128], F16, tag="ident")
            make_identity(nc, ident[:])

            def load_const(name, dram, shape, dtype):
                t = cpool.tile(shape, dtype, tag=name)
                nc.sync.dma_start(out=t[:], in_=dram.ap())
                return t

            w1t_s = load_const("w1t", w1t_d, [128, 128], F16)
            w2t_s = load_const("w2t", w2t_d, [128, 128], F16)
            w3t_s = load_const("w3t", w3t_d, [128, 1], F16)
            b1s_s = load_const("b1s", b1s_d, [128, 1], F32)
            b2s_s = load_const("b2s", b2s_d, [128, 1], F32)
            idx_s = {g: load_const(f"idx_{g}", idx_d[g], [128, nb * IC], I16)
                     for g in "eo"}
            w_s = {g: load_const(f"w_{g}", w_d[g], [128, tpg * 16], F16)
                   for g in "eo"}

            # Overlapping row view: row r = fp16 elems [r*256, r*256+512)
            tab_view = {g: bass.AP(tab_d[g].ap().tensor, 0,
                                   [[STEP, TAB_ROWS - 1], [1, ELEM]])
                        for g in "eo"}
            dma_sems = [nc.alloc_semaphore(f"gsem{q}") for q in range(n_queues)]

            qn = 0
            for gi, g in enumerate("eo"):
                goff = gi * tpg * 128
                for b in range(nb):
                    q = qn % n_queues
                    gt = gpool.tile([128, GB * 2, ELEM], F16, tag="gt")
                    if PREP_TRIGGER:
                        nc.gpsimd.dma_gather(
                            gt[:], tab_view[g],
                            idx_s[g][:, b * IC:(b + 1) * IC],
                            NUM_IDX, NUM_IDX, ELEM, elem_step=STEP,
                            single_packet=False, queue_num=q,
                            prepare_only=True, sem=dma_sems[q],
                        )
                        nc.gpsimd.trigger_dma(count=None, queue_num=q)
                    else:
                        nc.gpsimd.dma_gather(
                            gt[:], tab_view[g],
                            idx_s[g][:, b * IC:(b + 1) * IC],
                            NUM_IDX, NUM_IDX, ELEM, elem_step=STEP,
                            single_packet=False, queue_num=q,
                        )
                    qn += 1
                    # [128, tile(4), block(8 = y0a y0b y1a y1b | z...), 64, 2]
                    gv = gt[:].rearrange("p (t s) (h e two) -> p t (s h) e two",
                                         t=GB, h=4, two=2)
                    wb = w_s[g][:, b * 64:(b + 1) * 64].rearrange(
                        "p (t j two) -> p t j two", t=GB, two=2)
                    w8 = wb.unsqueeze(3).to_broadcast([128, GB, 8, 64, 2])

                    p8 = wpool.tile([128, GB * 8 * 128], F16, tag="p8")
                    p8v5 = p8[:].rearrange("p (t j e two) -> p t j e two",
                                           t=GB, j=8, two=2)
                    nc.vector.tensor_tensor(out=p8v5, in0=gv, in1=w8, op=mult)
                    p8v = p8[:].rearrange("p (t j e) -> p t j e", t=GB, j=8)
                    # pairwise tree: (j0+j1),(j2+j3),... -> then again
                    q4 = wpool.tile([128, GB * 4 * 128], F16, tag="q4")
                    q4v = q4[:].rearrange("p (t j e) -> p t j e", t=GB, j=4)
                    nc.vector.tensor_add(q4v, p8v[:, :, 0::2, :], p8v[:, :, 1::2, :])
                    q2 = wpool.tile([128, GB * 2 * 128], F16, tag="q2")
                    q2v = q2[:].rearrange("p (t j e) -> p t j e", t=GB, j=2)
                    nc.vector.tensor_add(q2v, q4v[:, :, 0::2, :], q4v[:, :, 1::2, :])
                    xy = q2v[:, :, 0, :]                       # [128, GB, 128]
                    xz = q2v[:, :, 1, :]

                    xz2 = wpool.tile([128, GB * 128], F16, tag="xz2")
                    xz2v = xz2[:].rearrange("p (t e) -> p t e", t=GB)
                    nc.scalar.square(xz2v, xz)
                    feat = wpool.tile([128, GB * 128], F16, tag="feat")
                    featv = feat[:].rearrange("p (t e) -> p t e", t=GB)
                    nc.vector.tensor_tensor(out=featv, in0=xy, in1=xz2v, op=mult)

                    ps_ft = ps_ft_pool.tile([128, GB * 128], F16, tag="ft")
                    for t in range(GB):
                        nc.tensor.transpose(
                            ps_ft[:, t * 128:(t + 1) * 128],
                            feat[:, t * 128:(t + 1) * 128],
                            ident[:],
                        )
                    ft = wpool.tile([128, GB * 128], F16, tag="ft_s")
                    nc.scalar.copy(ft[:], ps_ft[:])

                    ps1 = ps_mm_pool.tile([128, GB * 128], F32, tag="mm")
                    nc.tensor.matmul(ps1[:], w1t_s[:], ft[:], start=True, stop=True)
                    h1 = wpool.tile([128, GB * 128], F16, tag="h1")
                    nc.scalar.activation(h1[:], ps1[:], Sin, bias=b1s_s[:], scale=30.0)

                    ps2 = ps_mm_pool.tile([128, GB * 128], F32, tag="mm")
                    nc.tensor.matmul(ps2[:], w2t_s[:], h1[:], start=True, stop=True)
                    h2 = wpool.tile([128, GB * 128], F16, tag="h2")
                    nc.scalar.activation(h2[:], ps2[:], Sin, bias=b2s_s[:], scale=30.0)

                    ps3 = ps_mm_pool.tile([1, GB * 128], F32, tag="mm")
                    nc.tensor.matmul(ps3[:], w3t_s[:], h2[:], start=True, stop=True)
                    ob = wpool.tile([1, GB * 128], F32, tag="ob")
                    nc.scalar.copy(ob[:], ps3[:])
                    nc.sync.dma_start(
                        out=out_d.ap()[goff + b * 512: goff + (b + 1) * 512],
                        in_=ob[:],
                    )
    nc.compile()
    return nc


def prep_core(pts, tpg=TPG):
    """Host-side prep for one core's points -> device input tensors + orders."""
    nb = tpg // GB
    cap = tpg * 128
    IC = NUM_IDX // 16
    gx, gy, gz = pts[:, 0], pts[:, 1], pts[:, 2]
    ix = (gx + np.float32(1.0)) * np.float32(127.5)
    iy = (gy + np.float32(1.0)) * np.float32(127.5)
    iz = (gz + np.float32(1.0)) * np.float32(127.5)
    x0 = np.clip(np.floor(ix), 0, 254).astype(np.int32)
    y0 = np.clip(np.floor(iy), 0, 254).astype(np.int32)
    z0 = np.clip(np.floor(iz), 0, 254).astype(np.int32)
    wx1 = (ix - x0).astype(np.float32)
    wy1 = (iy - y0).astype(np.float32)
    wz1 = (iz - z0).astype(np.float32)
    wx0 = np.float32(1.0) - wx1
    wy0 = np.float32(1.0) - wy1
    wz0 = np.float32(1.0) - wz1

    par = (x0 & 1).astype(bool)
    # Sort each group by y-window patch index: descriptors then walk the
    # table near-sequentially (z-windows share the xp block), which lifts
    # SDMA gather throughput from random-read toward sequential.
    ykey = (x0 >> 1) * 256 + y0
    orders = []
    for mask in (~par, par):
        o = np.nonzero(mask)[0]
        orders.append(o[np.argsort(ykey[o], kind="stable")])
    outm = {}
    for gname, order in zip("eo", orders):
        ne = len(order)
        assert ne <= cap, f"group {gname} overflow: {ne} > {cap}"
        idx2 = np.zeros((cap, 2), np.int32)
        base = (x0[order] >> 1) * 256
        idx2[:ne, 0] = base + y0[order]
        idx2[:ne, 1] = base + z0[order]
        assert idx2.max() <= 32766
        w8 = np.zeros((cap, 8), np.float32)
        w8[:ne] = np.stack([
            wy0[order] * wx0[order], wy0[order] * wx1[order],
            wy1[order] * wx0[order], wy1[order] * wx1[order],
            wz0[order] * wx0[order], wz0[order] * wx1[order],
            wz1[order] * wx0[order], wz1[order] * wx1[order],
        ], axis=1)
        # idx stream -> [128, nb*IC] int16 tile (16-partition wrap, 8x replicated)
        a = idx2.reshape(nb, GB, 128, 2).transpose(0, 1, 3, 2)   # (b, t, s, p)
        stream = a.reshape(nb, NUM_IDX)
        t16 = stream.reshape(nb, NUM_IDX // 16, 16).transpose(0, 2, 1)  # (b, q, s)
        idx_dev = np.concatenate(
            [np.tile(t16[b], (8, 1)) for b in range(nb)], axis=1
        ).astype(np.int16)
        # duplicate each weight into an adjacent pair -> [128, tpg*8*2] f16
        w8h = w8.astype(np.float16)
        wpair = np.repeat(w8h.reshape(tpg, 128, 8), 2, axis=2)   # (t, p, 16)
        w_dev = np.ascontiguousarray(
            wpair.transpose(1, 0, 2).reshape(128, tpg * 16))
        outm[f"idx_{gname}"] = idx_dev
        outm[f"w_{gname}"] = w_dev
        outm[f"ord_{gname}"] = order
    return outm


def prep_shared(inputs):
    plane = np.asarray(inputs["plane_xy"], np.float32)[0]        # (C, H, W)
    hwc = np.ascontiguousarray(plane.transpose(1, 2, 0))         # (H, W, C)
    # T[xp, y, h, c] = p0[c, y, 2xp+h]
    tab_e = np.ascontiguousarray(
        hwc.reshape(256, 128, 2, 128).transpose(1, 0, 2, 3)
    ).reshape(-1).astype(np.float16)
    hwc_s = np.concatenate([hwc[:, 1:, :],
                            np.zeros((256, 1, 128), np.float32)], axis=1)
    tab_o = np.ascontiguousarray(
        hwc_s.reshape(256, 128, 2, 128).transpose(1, 0, 2, 3)
    ).reshape(-1).astype(np.float16)
    shared = {
        "tab_e": tab_e,
        "tab_o": tab_o,
        "w1t": np.ascontiguousarray(np.asarray(inputs["W1"], np.float32).T).astype(np.float16),
        "w2t": np.ascontiguousarray(np.asarray(inputs["W2"], np.float32).T).astype(np.float16),
        "w3t": np.ascontiguousarray(np.asarray(inputs["W3"], np.float32).T).astype(np.float16),
        "b1s": (np.float32(30.0) * np.asarray(inputs["b1"], np.float32)).reshape(128, 1),
        "b2s": (np.float32(30.0) * np.asarray(inputs["b2"], np.float32)).reshape(128, 1),
    }
    return shared


_NC_CACHE = {}


def get_nc(tpg=TPG):
    if tpg not in _NC_CACHE:
        _NC_CACHE[tpg] = build_nc(tpg)
    return _NC_CACHE[tpg]


LAST_RESULT = None


def kernel(_trace=False, **inputs):
    global LAST_RESULT
    from concourse.bass_utils import run_bass_kernel_spmd

    coords = np.asarray(inputs["coordinates"], np.float32).reshape(-1, 3)
    n = coords.shape[0]
    assert n == N_TOTAL, n
    shared = prep_shared(inputs)
    b3 = np.float32(np.asarray(inputs["b3"], np.float32).reshape(-1)[0])
    nc = get_nc()
    in_maps, metas = [], []
    for ci in range(N_CORES):
        pts = coords[ci * N_PER_CORE:(ci + 1) * N_PER_CORE]
        m = prep_core(pts)
        metas.append(m)
        in_maps.append({**shared,
                        "idx_e": m["idx_e"], "w_e": m["w_e"],
                        "idx_o": m["idx_o"], "w_o": m["w_o"]})
    res = run_bass_kernel_spmd(nc, in_maps, core_ids=list(range(N_CORES)),
                               trace=_trace)
    LAST_RESULT = res
    outs = []
    for ci in range(N_CORES):
        r = np.asarray(res.results[ci]["out"], np.float32)
        m = metas[ci]
        full = np.empty(N_PER_CORE, np.float32)
        full[m["ord_e"]] = r[:len(m["ord_e"])]
        full[m["ord_o"]] = r[PTS_PER_GROUP:PTS_PER_GROUP + len(m["ord_o"])]
        outs.append(full + b3)
    return np.concatenate(outs).reshape(1, N_TOTAL, 1)
